# revision 9
# baseline (speedup 1.0000x reference)
"""Trainium2 Bass kernel for LDPC sum-product BP decoding (nn_BP_Decoder).

Takes FULL unsharded inputs (llr_demapper [1024, 2040] plus Tanner-graph
index arrays), data-parallel over the batch axis across 8 NeuronCores
(128 batch rows per core = the SBUF partition count), returns the FULL
[1024, 2040] float32 output.

v2 consolidated layout: all three row-blocks of the (3,6)-regular code
share one edge tensor x_all [128, 3 blocks, 6 slots, 340 checks] (fp16).
The internal variable order p(v) = (v%6)*340 + v//6 makes block 0's
slot-major edge order coincide with v-space, so block 0 never crosses a
permutation.  Check->var runs consolidated across blocks: one 6120-wide
tanh, 12 slot-plane multiplies of 1020 columns each (vs 36x340 before),
two 6120-wide Ln ops (clip folded into the ACT affine), three 2040-wide
fp16 subtracts.  Var->check: fp16 v-space sums; blocks 1/2 cross via
GPSIMD local_scatter (~5 us each, overlapped with DVE/ACT).

The global sign flip of the reference (llr = -llr_demapper, out =
-llr_dec) cancels by oddness, so the kernel runs on llr_demapper
directly.  Accuracy vs reference: rel l2 ~1.7e-5 (fp16 transport).
"""
import functools
import numpy as np

import concourse.bacc as bacc
import concourse.tile as tile
import concourse.mybir as mybir
from concourse.tile_rust import add_dep_helper
from contextlib import ExitStack

F32 = mybir.dt.float32
F16 = mybir.dt.float16
I16 = mybir.dt.int16
AF = mybir.ActivationFunctionType
OP = mybir.AluOpType

N = 2040      # variables (and per-block edges)
NGRP = 340    # checks per block
DC = 6
N_CORES = 8
M_CLIP = float(np.float32(1.0) - np.float32(1e-7))


@functools.lru_cache(maxsize=2)
def _build_bp(nb_iter, num_devices=N_CORES):
    nc = bacc.Bacc("TRN2", target_bir_lowering=False, debug=False,
                   enable_asserts=False, num_devices=num_devices)
    xinit = nc.dram_tensor("xinit", [128, 3, DC, NGRP], F16,
                           kind="ExternalInput").ap()
    llr16 = nc.dram_tensor("llr16", [128, N], F16, kind="ExternalInput").ap()
    llr32 = nc.dram_tensor("llr32", [128, N], F32, kind="ExternalInput").ap()
    sidx = nc.dram_tensor("sidx", [128, 4 * N], I16, kind="ExternalInput").ap()
    out = nc.dram_tensor("out", [128, N], F32, kind="ExternalOutput").ap()

    with tile.TileContext(nc) as tc, ExitStack() as ctx:
        pool = ctx.enter_context(tc.tile_pool(name="p", bufs=1))

        x_all = pool.tile([128, 3, DC, NGRP], F16, tag="x_all", name="x_all")
        t_all = pool.tile([128, 3, DC, NGRP], F32, tag="t_all", name="t_all")
        ps_all = pool.tile([128, 3, DC, NGRP], F32, tag="ps_all", name="ps_all")
        ex_all = pool.tile([128, 3, DC, NGRP], F32, tag="ex_all", name="ex_all")
        cvph = pool.tile([128, 3, DC, NGRP], F16, tag="cvph", name="cvph")
        llr16s = pool.tile([128, N], F16, tag="llr16s", name="llr16s")
        llr32s = pool.tile([128, N], F32, tag="llr32s", name="llr32s")
        idx_s = pool.tile([128, 4 * N], I16, tag="idx_s", name="idx_s")
        cv1v = pool.tile([128, N], F16, tag="cv1v", name="cv1v")
        cv2v = pool.tile([128, N], F16, tag="cv2v", name="cv2v")
        a0h = pool.tile([128, N], F16, tag="a0h", name="a0h")
        uh = pool.tile([128, N], F16, tag="uh", name="uh")
        W1h = pool.tile([128, N], F16, tag="W1h", name="W1h")
        W2h = pool.tile([128, N], F16, tag="W2h", name="W2h")
        S = pool.tile([128, N], F32, tag="S", name="S")

        nc.sync.dma_start(x_all[:, :, :, :], xinit)
        nc.sync.dma_start(llr16s[:], llr16)
        nc.sync.dma_start(llr32s[:], llr32)
        nc.sync.dma_start(idx_s[:], sidx)
        ix_cv1 = idx_s[:, 0 * N:1 * N]
        ix_cv2 = idx_s[:, 1 * N:2 * N]
        ix_w1 = idx_s[:, 2 * N:3 * N]
        ix_w2 = idx_s[:, 3 * N:4 * N]

        def flat2040(ap4, b):
            return ap4[:, b, :, :].rearrange("a b c -> a (b c)")

        cv0f = flat2040(cvph, 0)
        cv1f = flat2040(cvph, 1)
        cv2f = flat2040(cvph, 2)
        x0f = flat2040(x_all, 0)
        x1f = flat2040(x_all, 1)
        x2f = flat2040(x_all, 2)

        def c2v():
            """tanh -> 12 wide slot-plane mults -> 2 lns -> 3 subs.

            (A per-block ln split was tried and REGRESSED 33.6 -> 42.3
            us/iter: six narrow ACT ops plus the extra WAR sync cost more
            than the chain shortening buys.)
            """
            nc.scalar.activation(t_all[:, :, :, :], x_all[:, :, :, :],
                                 AF.Tanh, scale=0.5)
            # pair-product exclude-self in 6 wide ops (same math as the
            # 12-op prefix/suffix scheme):
            #   q_p = t_{2p} * t_{2p+1}                 (1 op, 3060 cols)
            #   r_p = prod of the other two q's          (3 ops, 1020 cols)
            #   ex_{2p}   = t_{2p+1} * r_p  \  (2 ops, 3060 cols each,
            #   ex_{2p+1} = t_{2p}   * r_p  /   stride-2 plane views)
            v = nc.vector
            t_ev = t_all[:, :, 0:DC:2, :]
            t_od = t_all[:, :, 1:DC:2, :]
            q3 = ps_all[:, :, 0:3, :]
            r3 = ps_all[:, :, 3:6, :]
            q = [ps_all[:, :, p, :] for p in range(3)]
            r = [ps_all[:, :, 3 + p, :] for p in range(3)]
            v.tensor_tensor(q3, t_ev, t_od, OP.mult)
            v.tensor_tensor(r[0], q[1], q[2], OP.mult)
            v.tensor_tensor(r[1], q[0], q[2], OP.mult)
            v.tensor_tensor(r[2], q[0], q[1], OP.mult)
            v.tensor_tensor(ex_all[:, :, 0:DC:2, :], t_od, r3, OP.mult)
            v.tensor_tensor(ex_all[:, :, 1:DC:2, :], t_ev, r3, OP.mult)
            nc.scalar.activation(t_all[:, :, :, :], ex_all[:, :, :, :],
                                 AF.Ln, scale=M_CLIP, bias=1.0)
            nc.scalar.activation(ps_all[:, :, :, :], ex_all[:, :, :, :],
                                 AF.Ln, scale=-M_CLIP, bias=1.0)
            # subs: blocks 1 and 2 first (they gate the next scatters)
            nc.vector.tensor_tensor(cv1f, flat2040(t_all, 1),
                                    flat2040(ps_all, 1), OP.subtract)
            nc.vector.tensor_tensor(cv2f, flat2040(t_all, 2),
                                    flat2040(ps_all, 2), OP.subtract)
            nc.vector.tensor_tensor(cv0f, flat2040(t_all, 0),
                                    flat2040(ps_all, 0), OP.subtract)

        def scat(dst, src, ix):
            return nc.gpsimd.local_scatter(dst, src, ix, channels=128,
                                           num_elems=N, num_idxs=N)

        def chain_pool(insts):
            for a_, b_ in zip(insts[1:], insts):
                add_dep_helper(a_.ins, b_.ins, sync=False, reason="pool order")

        c2v()
        prev_scat = []
        for _ in range(nb_iter):
            s2 = scat(cv2v[:], cv2f, ix_cv2)
            nc.vector.tensor_tensor(a0h[:], llr16s[:], cv0f, OP.add)
            s1 = scat(cv1v[:], cv1f, ix_cv1)
            nc.vector.tensor_tensor(W1h[:], a0h[:], cv2v[:], OP.add)
            s3 = scat(x1f, W1h[:], ix_w1)
            nc.vector.tensor_tensor(W2h[:], a0h[:], cv1v[:], OP.add)
            nc.vector.tensor_tensor(uh[:], llr16s[:], cv1v[:], OP.add)
            s4 = scat(x2f, W2h[:], ix_w2)
            nc.vector.tensor_tensor(x0f, uh[:], cv2v[:], OP.add)
            chain_pool(prev_scat[-1:] + [s2, s1, s3, s4])
            prev_scat = [s4]
            c2v()
        s1 = scat(cv1v[:], cv1f, ix_cv1)
        s2 = scat(cv2v[:], cv2f, ix_cv2)
        chain_pool(prev_scat + [s1, s2])
        nc.vector.tensor_tensor(S[:], llr32s[:], cv0f, OP.add)
        nc.vector.tensor_tensor(S[:], S[:], cv1v[:], OP.add)
        nc.vector.tensor_tensor(S[:], S[:], cv2v[:], OP.add)
        nc.sync.dma_start(out, S[:])
    nc.compile()
    return nc


_active_builder = _build_bp


def _graph_indices(vn_msg_ind):
    """Internal order p(v) = (v%6)*340 + v//6; slot-major edge positions."""
    vg = np.asarray(vn_msg_ind, dtype=np.int64).reshape(N, 3)
    assert (vg[:, 0] == np.arange(N)).all(), "unexpected code structure"
    v = np.arange(N)
    pv = (v % DC) * NGRP + v // DC
    res = {"pv": pv}
    for b in (1, 2):
        e = vg[:, b] - b * N          # flat check-major edge pos of var v
        j = (e % DC) * NGRP + e // DC  # slot-major position
        ix_cv = np.empty(N, np.int64)
        ix_cv[j] = pv                  # dst[ix_cv[src]] = src
        ix_w = np.empty(N, np.int64)
        ix_w[pv] = j
        res[b] = (ix_cv, ix_w, j)
    return res


class _Runner:
    """jit-compiled PJRT executor for a prebuilt Bass module on 8 cores."""

    def __init__(self, nc):
        import jax
        from jax.sharding import Mesh, PartitionSpec
        from jax.experimental.shard_map import shard_map
        from concourse.bass2jax import (_bass_exec_p, install_neuronx_cc_hook,
                                        partition_id_tensor)
        install_neuronx_cc_hook()
        self.jax = jax
        partition_name = (nc.partition_id_tensor.name
                          if nc.partition_id_tensor else None)
        in_names, out_names, out_avals, zero_outs = [], [], [], []
        for alloc in nc.m.functions[0].allocations:
            if not isinstance(alloc, mybir.MemoryLocationSet):
                continue
            name = alloc.memorylocations[0].name
            if alloc.kind == "ExternalInput":
                if name != partition_name:
                    in_names.append(name)
            elif alloc.kind == "ExternalOutput":
                out_names.append(name)
                shape = tuple(alloc.tensor_shape)
                dtype = mybir.dt.np(alloc.dtype)
                out_avals.append(jax.core.ShapedArray(shape, dtype))
                zero_outs.append(np.zeros(shape, dtype))
        self.in_names, self.out_names = in_names, out_names
        self.out_avals, self.zero_outs = out_avals, zero_outs
        n_params, n_outs = len(in_names), len(out_avals)
        all_in = tuple(in_names + out_names
                       + ([partition_name] if partition_name else []))
        donate = tuple(range(n_params, n_params + n_outs))

        def _body(*args):
            operands = list(args)
            if partition_name is not None:
                operands.append(partition_id_tensor())
            return tuple(_bass_exec_p.bind(
                *operands, out_avals=tuple(out_avals), in_names=all_in,
                out_names=tuple(out_names), lowering_input_output_aliases=(),
                sim_require_finite=True, sim_require_nnan=True, nc=nc))

        devices = jax.devices()[:N_CORES]
        mesh = Mesh(np.asarray(devices), ("core",))
        self.fn = jax.jit(
            shard_map(_body, mesh=mesh,
                      in_specs=(PartitionSpec("core"),) * (n_params + n_outs),
                      out_specs=(PartitionSpec("core"),) * n_outs,
                      check_rep=False),
            donate_argnums=donate, keep_unused=True)

    def run(self, in_maps):
        per_core = [[np.asarray(m[n]) for n in self.in_names] for m in in_maps]
        args = [np.concatenate([per_core[c][i] for c in range(N_CORES)], axis=0)
                for i in range(len(self.in_names))]
        args += [np.zeros((N_CORES * z.shape[0], *z.shape[1:]), z.dtype)
                 for z in self.zero_outs]
        outs = self.fn(*[self.jax.numpy.asarray(a) for a in args])
        self.jax.block_until_ready(outs)
        return [{n: np.asarray(outs[i]).reshape(N_CORES, *self.out_avals[i].shape)[c]
                 for i, n in enumerate(self.out_names)} for c in range(N_CORES)]


_runner_cache = {}


def _get_runner(nb_iter):
    if nb_iter not in _runner_cache:
        _runner_cache[nb_iter] = _Runner(_build_bp(nb_iter))
    return _runner_cache[nb_iter]


def _prep_in_map(llr_slice, gi):
    """Per-core input dict from a [128, N] llr slice + graph indices."""
    pv = gi["pv"]
    ix_cv1, ix_w1, j1 = gi[1]
    ix_cv2, ix_w2, j2 = gi[2]
    llr_int = np.zeros((128, N), np.float32)
    llr_int[:, pv] = llr_slice
    xinit = np.empty((128, 3 * N), np.float32)
    xinit[:, 0:N] = llr_int
    xinit[:, N + j1] = llr_slice
    xinit[:, 2 * N + j2] = llr_slice
    sidx = np.concatenate([ix_cv1, ix_cv2, ix_w1, ix_w2]).astype(np.int16)
    return {
        "xinit": np.ascontiguousarray(
            xinit.astype(np.float16).reshape(128, 3, DC, NGRP)),
        "llr16": np.ascontiguousarray(llr_int.astype(np.float16)),
        "llr32": np.ascontiguousarray(llr_int),
        "sidx": np.ascontiguousarray(np.tile(sidx[None, :], (128, 1))),
    }


def kernel(llr_demapper, cn_msg_ind, vn_msg_ind, vn2cn_ind, cn_mask_ind,
           vn_mask_ind, edge_vn, nb_iter):
    llr = np.asarray(llr_demapper, dtype=np.float32)
    B = llr.shape[0]
    assert llr.shape == (B, N) and B % N_CORES == 0
    nb_iter = int(np.asarray(nb_iter))
    gi = _graph_indices(vn_msg_ind)

    rows = B // N_CORES
    assert rows == 128, "kernel is specialized for 128 batch rows per core"
    in_maps = [_prep_in_map(np.ascontiguousarray(llr[c * rows:(c + 1) * rows]),
                            gi) for c in range(N_CORES)]

    runner = _get_runner(nb_iter)
    res = runner.run(in_maps)
    S = np.concatenate([r["out"] for r in res], axis=0)
    return np.ascontiguousarray(S[:, gi["pv"]])
